# revision 24
# baseline (speedup 1.0000x reference)
"""Trainium2 Bass kernel for CenterWoParamMultiCosineLossV2.

Math (per sample b with label l):
    d_k   = 1 + <x_b, centers[l, k]>          k = 0..7
    value = (sum_k d_k^2) / (sum_k d_k)
    loss  = mean_b value

With u = sum_k <x_b, c_k> = <x_b, csum_l> and q = sum_k <x_b, c_k>^2:
    den = 8 + u,  num = 8 + 2u + q,  value = num / den

Strategy (loss is a mean -> permutation invariant):
  * Host sorts samples by label; each of the 8 cores takes 1024
    consecutive sorted samples, which span only ~13 classes.
  * Per core, a class table [512, nslot*9] holds each local class's 8
    centers + their sum (transposed).  One PE matmul computes
    S^T = table^T-applied scores [nslot*9, 1024] in fp32 (full
    precision is required: nearly-singular denominators amplify any
    lower-precision matmul far beyond the reference's fp32 envelope).
  * PE-transpose 128-sample blocks back to [128 samples, nslot*9],
    square + segment-reduce per class slot, select the sample's slot
    with a one-hot mask (fused multiply+reduce), then the num/den
    ratio per sample on DVE.
  * Each core returns its 1024 per-sample values; host sums and
    divides by 8192.
"""

import numpy as np
from contextlib import ExitStack

import concourse.bass as bass
import concourse.tile as tile
import concourse.mybir as mybir
from concourse import bass_utils
from concourse.masks import make_identity

# ---------------------------------------------------------------------------
# Workaround: this walrus build accepts only ONE sem-wait per instruction
# ("Too many sync wait commands"), but Tile freely attaches several waits at
# join points.  Post-pass: for any instruction with k>1 waits, hoist k-1 of
# them onto same-engine nops inserted immediately before it.  Tile's per-
# engine stream is a projection of one topological order, so a producer's
# trigger always precedes a consumer's wait and engine-level blocking cannot
# deadlock; sequential waits on monotonic sems == simultaneous waits.
# ---------------------------------------------------------------------------
_SPLIT_ID = [0]


def _split_multi_waits(nc):
    for f in nc.m.functions:
        for blk in f.blocks:
            insts = blk.instructions
            for idx in range(len(insts) - 1, -1, -1):
                inst = insts[idx]
                si = inst.sync_info
                waits = list(si.on_wait or []) if si is not None else []
                if len(waits) <= 1:
                    continue
                # For DMA instructions, keep a COMPUTE dependency on the
                # instruction (it rides the queue descriptor, so the DMA
                # pipeline pre-runs while parked on the sem) and hoist the
                # early-firing queue-guard sems onto the engine nop.
                if type(inst).__name__ == "InstDMACopy":
                    comp = [
                        w
                        for w in waits
                        if not str(w.ant_name or "").startswith("DMA")
                    ]
                    if comp:
                        keep = comp[-1]
                        waits = [w for w in waits if w is not keep] + [keep]
                inst.sync_info = mybir.SyncInfo(
                    on_wait=[waits[-1]], on_update=list(si.on_update or [])
                )
                for w in reversed(waits[:-1]):
                    _SPLIT_ID[0] += 1
                    nop = mybir.InstNoOp(
                        name=f"I-waitsplit-{_SPLIT_ID[0]}", ins=[], outs=[]
                    )
                    nop.engine = inst.engine
                    nop.sync_info = mybir.SyncInfo(on_wait=[w], on_update=[])
                    insts.insert(idx, nop)


def _rewrite_range_clears(nc):
    """This walrus build rejects the EVENT_SEMAPHORE_RANGE_CLEAR raw-ISA
    encoding ("ISA wrong length"); replace each with per-sem
    InstEventSemaphore sem-wr-imm 0 writes on the same engine."""
    for f in nc.m.functions:
        for blk in f.blocks:
            insts = blk.instructions
            for idx in range(len(insts) - 1, -1, -1):
                inst = insts[idx]
                if type(inst).__name__ != "InstISA":
                    continue
                s = str(inst)
                if "EVENT_SEMAPHORE_RANGE_CLEAR" not in s:
                    continue
                import re

                first = int(re.search(r"range_first=(\d+)", s).group(1))
                last = int(re.search(r"range_last=(\d+)", s).group(1))
                si = inst.sync_info
                waits = list(si.on_wait or []) if si is not None else []
                upds = list(si.on_update or []) if si is not None else []
                repl = []
                for j, sem in enumerate(range(first, last + 1)):
                    _SPLIT_ID[0] += 1
                    ev = mybir.InstEventSemaphore(
                        name=f"I-semclr-{_SPLIT_ID[0]}", ins=[], outs=[]
                    )
                    ev.engine = inst.engine
                    ev.sync_info = mybir.SyncInfo(
                        on_wait=waits if j == 0 else [],
                        on_update=[
                            mybir.SyncUpdate(
                                sync_type="semaphore",
                                id=sem,
                                update_mode="sem-wr-imm",
                                update_value=0,
                            )
                        ]
                        + (upds if j == (last - first) else []),
                    )
                    repl.append(ev)
                insts[idx : idx + 1] = repl


def _trim_tail(nc):
    """Exec time ends when the last engine halts.  The TileContext tail is
    [drain+barrier, 20 serial sem-clears on Pool, second barrier] -- ~2.5us
    after the output DMA completes.  Re-execution of the NEFF only needs the
    sems cleared before the tile block runs, so: clear them in the MAIN
    block instead (spread across engines, before the existing all-engine
    barrier that already orders engine start), and delete the tail clears +
    second barrier."""
    f = nc.m.functions[0]
    blocks = {b.name: b for b in f.blocks}
    main = blocks["main"]
    end = [b for n, b in blocks.items() if n.endswith("_end")][0]

    insts = end.instructions
    # find the Pool drain that precedes the semclear run (after barrier-1)
    clr_idx = [i for i, x in enumerate(insts) if x.name.startswith("I-semclr-")]
    if not clr_idx:
        return
    first, last = clr_idx[0], clr_idx[-1]
    clears = insts[first : last + 1]
    # everything after the clears is barrier-2 (+ its drains): delete; also
    # delete the clears and the extra Pool drain right before them
    start_del = first
    if start_del > 0 and type(insts[start_del - 1]).__name__ == "InstDrain":
        start_del -= 1
    del insts[start_del:]

    # re-insert clears near the start of main, round-robin across engines,
    # before the all-engine barrier (the barrier orders them vs tile work)
    m_insts = main.instructions
    # insertion point: before the first InstDrain (start of the barrier)
    ins_pt = next(
        (i for i, x in enumerate(m_insts) if type(x).__name__ == "InstDrain"),
        len(m_insts),
    )
    engines = [
        mybir.EngineType.Pool,
        mybir.EngineType.DVE,
        mybir.EngineType.Activation,
        mybir.EngineType.PE,
        mybir.EngineType.SP,
    ]
    for j, c in enumerate(clears):
        c.engine = engines[j % len(engines)]
        c.sync_info = mybir.SyncInfo(
            on_wait=[], on_update=list(c.sync_info.on_update or [])[:1]
        )
        m_insts.insert(ins_pt + j, c)

# ---------------------------------------------------------------------------

B, D, NCLS, KC = 8192, 512, 90, 8
NCORES, P = 8, 128
BC = B // NCORES          # samples per core
NBLK = BC // P            # 128-sample blocks per core
SW = KC + 1               # slot width: 8 center rows + 1 csum row
KCH = D // P              # contraction chunks
NTILE = 512               # moving-operand columns per matmul (fp32 max)

_BUILD_CACHE = {}


def _build(nslot, post_process=True):
    M = nslot * SW
    assert M <= 128, f"class slots {nslot} need {M} > 128 partitions"
    # The class table is padded to all 128 partitions: a [M<128, 512] PSUM
    # matmul output followed by a PE transpose-mode read hangs the device,
    # while the full-128 chain is fine (verified on HW); the pad rows are
    # zeros and their outputs are never read.
    f32 = mybir.dt.float32
    nc = bass.Bass("TRN2", target_bir_lowering=False, debug=False, num_devices=1)
    # xt is pre-chunked on the host: chunk (n, k) is a contiguous
    # [128, 512] block, so each chunk DMA is one linear 256 KiB read.
    xt_d = nc.dram_tensor("xt", [BC // NTILE, KCH, P, NTILE], f32, kind="ExternalInput")
    # partition-major: [128, KCH, 128] so each partition row is one
    # contiguous 2 KiB read
    ct_d = nc.dram_tensor("ct", [P, KCH, P], f32, kind="ExternalInput")
    e_d = nc.dram_tensor("e", [BC, nslot], f32, kind="ExternalInput")
    val_d = nc.dram_tensor("val", [P, NBLK], f32, kind="ExternalOutput")

    with tile.TileContext(nc) as tc:
        with ExitStack() as ctx:
            consts = ctx.enter_context(tc.tile_pool(name="consts", bufs=1))
            stp = ctx.enter_context(tc.tile_pool(name="stp", bufs=2))
            work = ctx.enter_context(tc.tile_pool(name="work", bufs=3))
            pst = ctx.enter_context(tc.tile_pool(name="pst", bufs=2, space="PSUM"))
            ptr = ctx.enter_context(tc.tile_pool(name="ptr", bufs=2, space="PSUM"))
            pwu = ctx.enter_context(tc.tile_pool(name="pwu", bufs=1, space="PSUM"))

            ident = consts.tile([P, P], f32)
            make_identity(nc, ident)
            # xt chunk (0,0) + the class table first (the first matmul
            # needs exactly those); spread issue cost across both HWDGE
            # engines (SP + ACT) and the gpsimd SWDGE ring so matmuls
            # start as early as possible and transfers overlap compute.
            xt_sb = consts.tile([P, KCH, BC], f32)
            xt_ap = xt_d.ap()
            ct_sb = consts.tile([P, KCH, P], f32)
            dma_engines = [nc.sync, nc.scalar, nc.gpsimd]
            di = 0
            for n in range(BC // NTILE):
                for k in range(KCH):
                    eng = dma_engines[di % len(dma_engines)]
                    di += 1
                    eng.dma_start(
                        out=xt_sb[:, k, n * NTILE : (n + 1) * NTILE],
                        in_=xt_ap[n, k],
                    )
                    if n == 0 and k == 0:
                        nc.scalar.dma_start(out=ct_sb, in_=ct_d.ap())
            e_sb = consts.tile([P, NBLK, nslot], f32)
            nc.sync.dma_start(
                out=e_sb, in_=e_d.ap().rearrange("(blk p) s -> p blk s", p=P)
            )

            # Dummy matmuls on the identity tile keep the PE busy while the
            # first xt chunk is in flight, so the HAM clock-gate is released
            # (1.2 -> 2.4 GHz) before the real matmuls start.
            wu_ps = pwu.tile([P, P], f32)
            for w in range(8):
                nc.tensor.matmul(wu_ps, ident, ident)

            nb = NTILE // P  # blocks per n-chunk
            for n in range(BC // NTILE):
                st_ps = pst.tile([P, NTILE], f32)
                for k in range(KCH):
                    nc.tensor.matmul(
                        st_ps,
                        ct_sb[:, k, :],
                        xt_sb[:, k, n * NTILE : (n + 1) * NTILE],
                        start=(k == 0),
                        stop=(k == KCH - 1),
                    )
                st_sb = stp.tile([P, NTILE], f32)
                nc.scalar.copy(st_sb, st_ps)
                # all transposes of this half land in ONE psum bank so the
                # square/reduce below batch over the whole half
                tr_ps = ptr.tile([P, nb * P], f32)
                for j in range(nb):
                    nc.tensor.matmul(
                        tr_ps[:, j * P : (j + 1) * P],
                        st_sb[:, j * P : (j + 1) * P],
                        ident,
                        is_transpose=True,
                        start=(j == 0),
                        stop=(j == nb - 1),
                        skip_group_check=True,
                    )
                tr4 = tr_ps.rearrange("p (j m) -> p j m", j=nb)[
                    :, :, 0:M
                ].rearrange("p j (s w) -> p j s w", w=SW)

                # per-half epilogue, all on DVE (no cross-engine hops):
                # q = sum_k s^2 over the sample's class slot; u likewise
                bs = slice(n * nb, (n + 1) * nb)
                zt = work.tile([P, nb, nslot, KC], f32)
                nc.scalar.activation(
                    zt, tr4[:, :, :, 0:KC], mybir.ActivationFunctionType.Square
                )
                q14 = work.tile([P, nb, nslot], f32)
                nc.vector.reduce_sum(q14, zt, axis=mybir.AxisListType.X)
                mq = work.tile([P, nb, nslot], f32)
                nc.vector.tensor_mul(mq, q14, e_sb[:, bs, :])
                qcol = work.tile([P, nb], f32)
                nc.vector.reduce_sum(qcol, mq, axis=mybir.AxisListType.X)
                mu = work.tile([P, nb, nslot], f32)
                nc.vector.tensor_mul(mu, tr4[:, :, :, KC], e_sb[:, bs, :])
                ucol = work.tile([P, nb], f32)
                nc.vector.reduce_sum(ucol, mu, axis=mybir.AxisListType.X)

                # den = u + 8; num = 2u + (q + 8); val = num / den
                den = work.tile([P, nb], f32)
                nc.vector.scalar_tensor_tensor(
                    den, ucol, 8.0, ucol,
                    op0=mybir.AluOpType.add, op1=mybir.AluOpType.bypass,
                )
                rde = work.tile([P, nb], f32)
                nc.vector.reciprocal(rde, den)
                qp8 = work.tile([P, nb], f32)
                nc.vector.scalar_tensor_tensor(
                    qp8, qcol, 8.0, qcol,
                    op0=mybir.AluOpType.add, op1=mybir.AluOpType.bypass,
                )
                num = work.tile([P, nb], f32)
                nc.vector.scalar_tensor_tensor(
                    num, ucol, 2.0, qp8,
                    op0=mybir.AluOpType.mult, op1=mybir.AluOpType.add,
                )
                val = work.tile([P, nb], f32)
                nc.vector.tensor_mul(val, num, rde)
                nc.sync.dma_start(out=val_d.ap()[:, bs], in_=val)
    if post_process:
        _rewrite_range_clears(nc)
        _trim_tail(nc)
        _split_multi_waits(nc)
    return nc


def _prep_in_maps(x, centers, labels):
    x = np.ascontiguousarray(np.asarray(x, dtype=np.float32))
    centers = np.asarray(centers, dtype=np.float32)
    labels = np.asarray(labels).astype(np.int64)
    order = np.argsort(labels, kind="stable")
    xs = x[order]
    ls = labels[order]

    core_classes = [np.unique(ls[i * BC : (i + 1) * BC]) for i in range(NCORES)]
    nslot = max(len(c) for c in core_classes)

    in_maps = []
    for i in range(NCORES):
        sl = slice(i * BC, (i + 1) * BC)
        # chunk-contiguous layout [n, k, 128, 512] (see _build)
        xT = np.ascontiguousarray(
            xs[sl].T.reshape(KCH, P, BC // NTILE, NTILE).transpose(2, 0, 1, 3)
        )
        cls = core_classes[i]
        ct = np.zeros((D, 128), np.float32)
        for j, c in enumerate(cls):
            ct[:, j * SW : j * SW + KC] = centers[c].T
            ct[:, j * SW + KC] = centers[c].sum(axis=0)
        ct = np.ascontiguousarray(ct.reshape(KCH, P, P).transpose(1, 0, 2))
        slot_of = {c: j for j, c in enumerate(cls)}
        e = np.zeros((BC, nslot), np.float32)
        e[np.arange(BC), [slot_of[c] for c in ls[sl]]] = 1.0
        in_maps.append({"xt": xT, "ct": ct, "e": e})
    return nslot, in_maps


def kernel(x, centers, labels, _trace=False):
    nslot, in_maps = _prep_in_maps(x, centers, labels)
    if nslot not in _BUILD_CACHE:
        _BUILD_CACHE[nslot] = _build(nslot)
    nc = _BUILD_CACHE[nslot]
    res = bass_utils.run_bass_kernel_spmd(
        nc, in_maps, core_ids=list(range(NCORES)), trace=_trace
    )
    total = 0.0
    for r in res.results:
        total += r["val"].astype(np.float64).sum()
    out = np.float32(total / B)
    if _trace:
        return out, res
    return out
